# revision 29
# baseline (speedup 1.0000x reference)
"""Trainium2 Bass kernel for nn_CBAMSpaceMask (CBAM spatial mask over T timestep blocks).

Math per timestep block t (3 channels):
  mx_c = maxpool3x3(x_c)          (stride 1, replicate-ish pad == clip)
  av_c = avgpool3x3(x_c)/9        (zero pad, count_include_pad)
  y_t  = sum_c conv3x3(wM_c, mx_c) + conv3x3(wA_c, bv(bh(x_c)))/9 + b
  out[3t+c] = sigmoid(leakyrelu(y_t))          (broadcast over c)

v2 design (per core = 1 batch):
  - input loaded from HBM exactly once (f32->bf16 casting SWDGE), 4 blocks x
    12 planes x 2 row-halves
  - hmax3/hsum3 on DVE (free-dim, bf16 2x); vertical 3-max via SBUF->SBUF
    partition-shift DMAs (HWDGE) + 2 DVE max ops
  - conv as banded-Toeplitz matmuls, out-row chunks A=[0,126) B'=[126,250)
    C=[250,256); B' reads a shift-copied frame (rows 124..251); C batched
    over plane-pairs into K=96 block-diagonal matmuls
  - 2 timesteps per matmul (N=508) with mat-outer order for LDWEIGHTS reuse
  - epilogue: ACT Lrelu(ps+bias, alpha=.01) -> ACT Sigmoid -> bf16 sg; one
    casting+3-channel-broadcast SWDGE DMA per (chunk, block) writes f32 out
Sharding: pure data parallel, batch dim across 8 cores.
"""
import sys

sys.path.insert(0, "/opt/trn_rl_repo")

import numpy as np
import ml_dtypes
from contextlib import ExitStack

import concourse.bass as bass
import concourse.tile as tile
from concourse import bacc, mybir
from concourse.bass_utils import run_bass_kernel_spmd

F32 = mybir.dt.float32
BF16 = mybir.dt.bfloat16

B, CTOT, H, W = 8, 48, 256, 256
T = 16
N_CORES = 8
PLANES = CTOT
NBLK = 4            # 12 planes (4 timesteps) per block
BP = 12             # planes per block
# out-row chunks: A rows [0,126), B' rows [126,250) from frame rows [124,252), C rows [250,256)
MA, MB, MC = 126, 124, 6
NMAT = 39           # 18 A-mats + 18 B'-mats + 3 C-mats

_cache = {}


def _mat_index(path, c, kw):
    return (path * 3 + c) * 3 + kw


def _build_ops(conv_w):
    """op[path][c][kw] = [256, 256] f64: y[m] = sum_r op[m, r] * P[r] for one kw tap.
    path 0: max (P = maxpooled plane), path 1: avg (P = horizontal box sum)."""
    w = conv_w[0].astype(np.float64)  # [6, 3, 3]
    Bv = np.zeros((H, H))
    for i in (-1, 0, 1):
        Bv += np.eye(H, k=i)
    ops = {}
    for path in range(2):
        for c in range(3):
            k2d = w[2 * c] if path == 0 else w[2 * c + 1]
            for kw in range(3):
                op = np.zeros((H, H))
                for kh in range(3):
                    op += k2d[kh, kw] * np.eye(H, k=kh - 1)
                if path == 1:
                    op = (op @ Bv) / 9.0
                ops[(path, c, kw)] = op
    return ops


def _build_stack(conv_w):
    """lhsT stack [128, NMAT, 128] f32->bf16.
    mats 0..17: chunk A  lhsT[r, m] = op[m, r],        r in [0,128), m in [0,126)
    mats 18..35: chunk B' lhsT[r, m] = op[126+m, 124+r], r in [0,128), m in [0,124)
    mats 36..38 (kw): C   lhsT[48*path+8j+r, 6dt+mm] = op[250+mm, 248+r]
                          for plane-in-pair j (t-half dt=j//3, c=j%3), r in [0,8)
    """
    ops = _build_ops(conv_w)
    stack = np.zeros((128, NMAT, 128), dtype=np.float64)
    for path in range(2):
        for c in range(3):
            for kw in range(3):
                op = ops[(path, c, kw)]
                mat = _mat_index(path, c, kw)
                stack[0:128, mat, 0:MA] = op[0:MA, 0:128].T
                stack[0:128, 18 + mat, 0:MB] = op[126:126 + MB, 124:252].T
    for kw in range(3):
        mat = 36 + kw
        for path in range(2):
            for j in range(6):
                dt, c = j // 3, j % 3
                op = ops[(path, c, kw)]
                for r in range(8):
                    for mm in range(MC):
                        stack[48 * path + 8 * j + r, mat, 6 * dt + mm] = \
                            op[250 + mm, 248 + r]
    return stack.astype(ml_dtypes.bfloat16)


def _build_program():
    nc = bacc.Bacc("TRN2", target_bir_lowering=False, debug=False, enable_asserts=False)
    x_ap = nc.dram_tensor("x", [PLANES, H, W], F32, kind="ExternalInput").ap()
    cst_ap = nc.dram_tensor("cst", [128, NMAT, 128], BF16, kind="ExternalInput").ap()
    bias_ap = nc.dram_tensor("bias", [128, 1], F32, kind="ExternalInput").ap()
    out_ap = nc.dram_tensor("out", [PLANES, H, W], F32, kind="ExternalOutput").ap()

    MAXOP = mybir.AluOpType.max
    ADDOP = mybir.AluOpType.add
    LRELU = mybir.ActivationFunctionType.Lrelu
    SIGMOID = mybir.ActivationFunctionType.Sigmoid

    # mat order within a chunk: a kw=1 mat first (full-width psum init)
    MATS = [(path, c, kw) for kw in (1, 0, 2) for path in range(2) for c in range(3)]

    with tile.TileContext(nc) as tc, ExitStack() as ctx:
        const_pool = ctx.enter_context(tc.tile_pool(name="const", bufs=1))
        ld_pool = ctx.enter_context(tc.tile_pool(name="loads", bufs=2))
        hm_pool = ctx.enter_context(tc.tile_pool(name="hmhs", bufs=2))
        ud_pool = ctx.enter_context(tc.tile_pool(name="ud", bufs=2))
        frm_pool = ctx.enter_context(tc.tile_pool(name="frames", bufs=2))
        sg_pool = ctx.enter_context(tc.tile_pool(name="sg", bufs=2))
        v_pool = ctx.enter_context(tc.tile_pool(name="vepi", bufs=2))
        psum_pool = ctx.enter_context(tc.tile_pool(name="psum", bufs=6, space="PSUM"))
        psc_pool = ctx.enter_context(tc.tile_pool(name="psc", bufs=1, space="PSUM"))
        dscr_pool = ctx.enter_context(
            tc.tile_pool(name="dscr", bufs=1, space=bass.MemorySpace.DRAM))

        cst = const_pool.tile([128, NMAT, 128], BF16, tag="cst")
        nc.sync.dma_start(out=cst[:], in_=cst_ap)
        bias = const_pool.tile([128, 1], F32, tag="bias")
        nc.sync.dma_start(out=bias[:], in_=bias_ap)

        # C-chunk rhs: partition k = 48*path + 8*j + r, free = (pair, w);
        # gathered via DRAM round-trip (SBUF DMA APs need contiguous partitions)
        crhs = const_pool.tile([96, 8, W], BF16, tag="crhs")
        cscr = dscr_pool.tile([NBLK, 2, 8, BP, W], BF16, tag="cscr")
        # C psums: 6 pairs per tile (3 bases x 2 free slots)
        ps_c0 = psc_pool.tile([128, 2, W], F32, tag="psc0")
        ps_c1 = psc_pool.tile([128, 2, W], F32, tag="psc1")
        ps_cs = (ps_c0, ps_c1)

        def psc_slice(p, lo, hi):
            rem = p % 6
            q = 32 * (rem // 2)
            return ps_cs[p // 6][q:q + 12, rem % 2, lo:hi]

        def hpool(out_t, in_t, op):
            """3-tap horizontal (free-dim) max/sum, zero/clip pad."""
            nc.vector.tensor_tensor(out=out_t[:, :, 0:W - 1], in0=in_t[:, :, 0:W - 1],
                                    in1=in_t[:, :, 1:W], op=op)
            nc.vector.tensor_copy(out_t[:, :, W - 1:W], in_t[:, :, W - 1:W])
            nc.vector.tensor_tensor(out=out_t[:, :, 1:W], in0=out_t[:, :, 1:W],
                                    in1=in_t[:, :, 0:W - 1], op=op)

        # ---- software-pipelined emission: pools(b+1) are emitted before
        # vmax(b) so the DVE never idles at its FIFO head waiting on shift DMAs
        def stage1(b):
            pl0 = BP * b
            xs = {}
            for h, r0 in ((0, 0), (1, 128)):
                X = ld_pool.tile([128, BP, W], BF16, tag=f"x{h}", name=f"x_{b}_{h}")
                nc.gpsimd.dma_start(
                    out=X[:], in_=x_ap[pl0:pl0 + BP, r0:r0 + 128, :].transpose([1, 0, 2]))
                xs[h] = X
            hms, hss = {}, {}
            for h in (0, 1):
                hm = hm_pool.tile([128, BP, W], BF16, tag=f"hm{h}", name=f"hm_{b}_{h}")
                hpool(hm, xs[h], MAXOP)
                hs = hm_pool.tile([128, BP, W], BF16, tag=f"hs{h}", name=f"hs_{b}_{h}")
                hpool(hs, xs[h], ADDOP)
                hms[h], hss[h] = hm, hs
            # shift copies can be emitted now (they only need hm)
            uds = {}
            for h in (0, 1):
                U = ud_pool.tile([128, BP, W], BF16, tag=f"u{h}", name=f"u_{b}_{h}")
                nc.gpsimd.dma_start(out=U[0:127], in_=hms[h][1:128])
                if h == 0:
                    nc.sync.dma_start(out=U[127:128], in_=hms[1][0:1])
                else:
                    nc.sync.dma_start(out=U[127:128], in_=hms[1][127:128])
                D = ud_pool.tile([128, BP, W], BF16, tag=f"d{h}", name=f"d_{b}_{h}")
                nc.gpsimd.dma_start(out=D[1:128], in_=hms[h][0:127])
                if h == 0:
                    nc.sync.dma_start(out=D[0:1], in_=hms[0][0:1])
                else:
                    nc.sync.dma_start(out=D[0:1], in_=hms[0][127:128])
                uds[h] = (U, D)
            return xs, hms, hss, uds

        def stage2(b, st):
            xs, hms, hss, uds = st
            # vmax in place: hm becomes the pooled max tile
            mxs = hms
            for h in (0, 1):
                U, D = uds[h]
                nc.vector.tensor_tensor(out=hms[h][:], in0=hms[h][:], in1=U[:], op=MAXOP)
                nc.vector.tensor_tensor(out=hms[h][:], in0=hms[h][:], in1=D[:], op=MAXOP)

            # B' frames (rows 124..251)
            mxp = frm_pool.tile([128, BP, W], BF16, tag="mxp", name=f"mxp_{b}")
            nc.sync.dma_start(out=mxp[0:4], in_=mxs[0][124:128])
            nc.gpsimd.dma_start(out=mxp[4:128], in_=mxs[1][0:124])
            hsp = frm_pool.tile([128, BP, W], BF16, tag="hsp", name=f"hsp_{b}")
            nc.sync.dma_start(out=hsp[0:4], in_=hss[0][124:128])
            nc.gpsimd.dma_start(out=hsp[4:128], in_=hss[1][0:124])

            # C-chunk gathers: pooled rows 248..255 -> DRAM -> packed crhs
            nc.sync.dma_start(out=cscr[b, 0], in_=mxs[1][120:128, :, :])
            nc.sync.dma_start(out=cscr[b, 1], in_=hss[1][120:128, :, :])
            for p_loc in range(2):
                p = 2 * b + p_loc
                for pa in range(2):
                    nc.sync.dma_start(
                        out=crhs[48 * pa:48 * pa + 48, p, :],
                        in_=cscr[b, pa, :, 6 * p_loc:6 * p_loc + 6, :].transpose([1, 0, 2]))
            return mxs, hss, mxp, hsp

        def stage3(b, st2):
            mxs, hss, mxp, hsp = st2
            pl0 = BP * b
            # ---- conv: chunks A and B', 2 timestep-pairs per block
            epi = []
            for chunk in range(2):
                M = MA if chunk == 0 else MB
                matbase = 18 * chunk
                pss = [psum_pool.tile([128, 2, W], F32, tag="ps", name=f"ps_{b}_{chunk}_{pp}")
                       for pp in range(2)]
                srcs = (mxs[0], hss[0]) if chunk == 0 else (mxp, hsp)
                n = len(MATS)
                for i, (path, c, kw) in enumerate(MATS):
                    mat = matbase + _mat_index(path, c, kw)
                    s = kw - 1
                    lo, hi = max(0, -s), W - max(0, s)
                    lhsT = cst[0:128, mat, 0:M]
                    # rhs planes (3*tt+c) for tt in {2p, 2p+1}: view (tt, c)
                    rsrc = srcs[path][:].rearrange("k (tt c) w -> k tt c w", tt=4, c=3)
                    for p_loc in range(2):
                        rhs = rsrc[0:128, 2 * p_loc:2 * p_loc + 2, c, lo + s:hi + s]
                        nc.tensor.matmul(pss[p_loc][0:M, :, lo:hi], lhsT, rhs,
                                         start=(i == 0), stop=(i == n - 1))
                sg = sg_pool.tile([126, 4, W], F32, tag=f"sg{chunk}")
                epi.append((chunk, M, pss, sg))
            # epilogue: all Lrelus, then all Sigmoids (avoid ACT table thrash)
            vs = {}
            for chunk, M, pss, sg in epi:
                for p_loc in range(2):
                    v = v_pool.tile([126, 2, W], BF16, tag=f"v{chunk}{p_loc}",
                                    name=f"v_{b}_{chunk}_{p_loc}")
                    nc.scalar.activation(v[0:M], pss[p_loc][0:M],
                                         LRELU, bias=bias[0:M], scale=1.0, alpha=0.01)
                    vs[(chunk, p_loc)] = v
            for chunk, M, pss, sg in epi:
                for p_loc in range(2):
                    nc.scalar.activation(sg[0:M, 2 * p_loc:2 * p_loc + 2, :],
                                         vs[(chunk, p_loc)][0:M], SIGMOID)
            for chunk, M, pss, sg in epi:
                r0 = 0 if chunk == 0 else MA
                dst = out_ap[pl0:pl0 + BP, r0:r0 + M, :].rearrange(
                    "(t c) m w -> m t c w", t=4, c=3)
                for ch in range(3):
                    nc.scalar.dma_start(out=dst[:, :, ch, :], in_=sg[0:M])

            # C matmuls for this block's two pairs
            for p_loc in range(2):
                p = 2 * b + p_loc
                for i, kw in enumerate((1, 0, 2)):
                    s = kw - 1
                    lo, hi = max(0, -s), W - max(0, s)
                    nc.tensor.matmul(psc_slice(p, lo, hi),
                                     cst[0:96, 36 + kw, 0:12],
                                     crhs[0:96, p, lo + s:hi + s],
                                     start=(i == 0), stop=(i == 2))

        st1 = {0: stage1(0)}
        st1[1] = stage1(1)
        st2s = {}
        for b in range(NBLK):
            if b + 2 < NBLK:
                st1[b + 2] = stage1(b + 2)
            st2s[b] = stage2(b, st1.pop(b))
            stage3(b, st2s.pop(b))

        # C epilogue + output (rows 250..255, all t).  ACT needs 32-aligned
        # partition bases -> process the (base, slot) psc layout wholesale,
        # then write out directly with per-(pair, channel) DMAs.
        outv = out_ap.rearrange("(pl three) h w -> pl three h w", pl=16, three=3)
        for ti in range(2):
            vca = const_pool.tile([96, 2, W], BF16, tag=f"vca{ti}", name=f"vca{ti}")
            nc.scalar.activation(vca[:], ps_cs[ti][0:96], LRELU,
                                 bias=bias[0:96], scale=1.0, alpha=0.01)
            sga = const_pool.tile([96, 2, W], F32, tag=f"sga{ti}", name=f"sga{ti}")
            nc.scalar.activation(sga[:], vca[:], SIGMOID)
            for u in range(3 if ti == 0 else 1):
                for slot in range(2):
                    p = 6 * ti + 2 * u + slot
                    for ch in range(3):
                        nc.sync.dma_start(
                            out=outv[2 * p:2 * p + 2, ch, 250:256, :],
                            in_=sga[32 * u:32 * u + 12, slot, :])

    nc.compile()
    return nc


def kernel(input_tensor, conv_w, conv_b):
    input_tensor = np.ascontiguousarray(np.asarray(input_tensor, dtype=np.float32))
    conv_w = np.asarray(conv_w, dtype=np.float32)
    conv_b = np.asarray(conv_b, dtype=np.float32)

    if "nc" not in _cache:
        _cache["nc"] = _build_program()
    nc = _cache["nc"]

    stack = _build_stack(conv_w)
    bias_vec = np.full((128, 1), conv_b[0], dtype=np.float32)
    in_maps = [
        {"x": input_tensor[i], "cst": stack, "bias": bias_vec}
        for i in range(N_CORES)
    ]
    res = run_bass_kernel_spmd(nc, in_maps, list(range(N_CORES)))
    out = np.stack([res.results[i]["out"] for i in range(N_CORES)], axis=0)
    return out.astype(np.float32)


if __name__ == "__main__":
    rng = np.random.default_rng(0)
    x = rng.standard_normal((B, CTOT, H, W), dtype=np.float32)
    cw = rng.uniform(-0.1, 0.1, (1, 6, 3, 3)).astype(np.float32)
    cb = np.array([0.01], dtype=np.float32)
    o = kernel(x, cw, cb)
    print(o.shape, o.dtype)


# revision 30
# speedup vs baseline: 1.0295x; 1.0295x over previous
"""Trainium2 Bass kernel for nn_CBAMSpaceMask (CBAM spatial mask over T timestep blocks).

Math per timestep block t (3 channels):
  mx_c = maxpool3x3(x_c)          (stride 1, replicate-ish pad == clip)
  av_c = avgpool3x3(x_c)/9        (zero pad, count_include_pad)
  y_t  = sum_c conv3x3(wM_c, mx_c) + conv3x3(wA_c, bv(bh(x_c)))/9 + b
  out[3t+c] = sigmoid(leakyrelu(y_t))          (broadcast over c)

v2 design (per core = 1 batch):
  - input loaded from HBM exactly once (f32->bf16 casting SWDGE), 4 blocks x
    12 planes x 2 row-halves
  - hmax3/hsum3 on DVE (free-dim, bf16 2x); vertical 3-max via SBUF->SBUF
    partition-shift DMAs (HWDGE) + 2 DVE max ops
  - conv as banded-Toeplitz matmuls, out-row chunks A=[0,126) B'=[126,250)
    C=[250,256); B' reads a shift-copied frame (rows 124..251); C batched
    over plane-pairs into K=96 block-diagonal matmuls
  - 2 timesteps per matmul (N=508) with mat-outer order for LDWEIGHTS reuse
  - epilogue: ACT Lrelu(ps+bias, alpha=.01) -> ACT Sigmoid -> bf16 sg; one
    casting+3-channel-broadcast SWDGE DMA per (chunk, block) writes f32 out
Sharding: pure data parallel, batch dim across 8 cores.
"""
import sys

sys.path.insert(0, "/opt/trn_rl_repo")

import numpy as np
import ml_dtypes
from contextlib import ExitStack

import concourse.bass as bass
import concourse.tile as tile
from concourse import bacc, mybir
from concourse.bass_utils import run_bass_kernel_spmd

F32 = mybir.dt.float32
BF16 = mybir.dt.bfloat16

B, CTOT, H, W = 8, 48, 256, 256
T = 16
N_CORES = 8
PLANES = CTOT
NBLK = 4            # 12 planes (4 timesteps) per block
BP = 12             # planes per block
# out-row chunks: A rows [0,126), B' rows [126,250) from frame rows [124,252), C rows [250,256)
MA, MB, MC = 126, 124, 6
NMAT = 39           # 18 A-mats + 18 B'-mats + 3 C-mats

_cache = {}


def _mat_index(path, c, kw):
    return (path * 3 + c) * 3 + kw


def _build_ops(conv_w):
    """op[path][c][kw] = [256, 256] f64: y[m] = sum_r op[m, r] * P[r] for one kw tap.
    path 0: max (P = maxpooled plane), path 1: avg (P = horizontal box sum)."""
    w = conv_w[0].astype(np.float64)  # [6, 3, 3]
    Bv = np.zeros((H, H))
    for i in (-1, 0, 1):
        Bv += np.eye(H, k=i)
    ops = {}
    for path in range(2):
        for c in range(3):
            k2d = w[2 * c] if path == 0 else w[2 * c + 1]
            for kw in range(3):
                op = np.zeros((H, H))
                for kh in range(3):
                    op += k2d[kh, kw] * np.eye(H, k=kh - 1)
                if path == 1:
                    op = (op @ Bv) / 9.0
                ops[(path, c, kw)] = op
    return ops


def _build_stack(conv_w):
    """lhsT stack [128, NMAT, 128] f32->bf16.
    mats 0..17: chunk A  lhsT[r, m] = op[m, r],        r in [0,128), m in [0,126)
    mats 18..35: chunk B' lhsT[r, m] = op[126+m, 124+r], r in [0,128), m in [0,124)
    mats 36..38 (kw): C   lhsT[48*path+8j+r, 6dt+mm] = op[250+mm, 248+r]
                          for plane-in-pair j (t-half dt=j//3, c=j%3), r in [0,8)
    """
    ops = _build_ops(conv_w)
    stack = np.zeros((128, NMAT, 128), dtype=np.float64)
    for path in range(2):
        for c in range(3):
            for kw in range(3):
                op = ops[(path, c, kw)]
                mat = _mat_index(path, c, kw)
                stack[0:128, mat, 0:MA] = op[0:MA, 0:128].T
                stack[0:128, 18 + mat, 0:MB] = op[126:126 + MB, 124:252].T
    for kw in range(3):
        mat = 36 + kw
        for path in range(2):
            for j in range(6):
                dt, c = j // 3, j % 3
                op = ops[(path, c, kw)]
                for r in range(8):
                    for mm in range(MC):
                        stack[48 * path + 8 * j + r, mat, 6 * dt + mm] = \
                            op[250 + mm, 248 + r]
    return stack.astype(ml_dtypes.bfloat16)


def _build_program():
    nc = bacc.Bacc("TRN2", target_bir_lowering=False, debug=False, enable_asserts=False)
    x_ap = nc.dram_tensor("x", [PLANES, H, W], F32, kind="ExternalInput").ap()
    cst_ap = nc.dram_tensor("cst", [128, NMAT, 128], BF16, kind="ExternalInput").ap()
    bias_ap = nc.dram_tensor("bias", [128, 1], F32, kind="ExternalInput").ap()
    out_ap = nc.dram_tensor("out", [PLANES, H, W], F32, kind="ExternalOutput").ap()

    MAXOP = mybir.AluOpType.max
    ADDOP = mybir.AluOpType.add
    LRELU = mybir.ActivationFunctionType.Lrelu
    SIGMOID = mybir.ActivationFunctionType.Sigmoid

    # mat order within a chunk: a kw=1 mat first (full-width psum init)
    MATS = [(path, c, kw) for kw in (1, 0, 2) for path in range(2) for c in range(3)]

    with tile.TileContext(nc) as tc, ExitStack() as ctx:
        const_pool = ctx.enter_context(tc.tile_pool(name="const", bufs=1))
        ld_pool = ctx.enter_context(tc.tile_pool(name="loads", bufs=2))
        hm_pool = ctx.enter_context(tc.tile_pool(name="hmhs", bufs=2))
        ud_pool = ctx.enter_context(tc.tile_pool(name="ud", bufs=2))
        frm_pool = ctx.enter_context(tc.tile_pool(name="frames", bufs=2))
        sg_pool = ctx.enter_context(tc.tile_pool(name="sg", bufs=2))
        v_pool = ctx.enter_context(tc.tile_pool(name="vepi", bufs=2))
        psum_pool = ctx.enter_context(tc.tile_pool(name="psum", bufs=6, space="PSUM"))
        psc_pool = ctx.enter_context(tc.tile_pool(name="psc", bufs=1, space="PSUM"))
        dscr_pool = ctx.enter_context(
            tc.tile_pool(name="dscr", bufs=1, space=bass.MemorySpace.DRAM))

        cst = const_pool.tile([128, NMAT, 128], BF16, tag="cst")
        nc.sync.dma_start(out=cst[:], in_=cst_ap)
        bias = const_pool.tile([128, 1], F32, tag="bias")
        nc.sync.dma_start(out=bias[:], in_=bias_ap)

        # C-chunk rhs: partition k = 48*path + 8*j + r, free = (pair, w);
        # gathered via DRAM round-trip (SBUF DMA APs need contiguous partitions)
        crhs = const_pool.tile([96, 8, W], BF16, tag="crhs")
        cscr = dscr_pool.tile([NBLK, 2, 8, BP, W], BF16, tag="cscr")
        # C psums: 6 pairs per tile (3 bases x 2 free slots)
        ps_c0 = psc_pool.tile([128, 2, W], F32, tag="psc0")
        ps_c1 = psc_pool.tile([128, 2, W], F32, tag="psc1")
        ps_cs = (ps_c0, ps_c1)

        def psc_slice(p, lo, hi):
            rem = p % 6
            q = 32 * (rem // 2)
            return ps_cs[p // 6][q:q + 12, rem % 2, lo:hi]

        def hpool(out_t, in_t, op):
            """3-tap horizontal (free-dim) max/sum, zero/clip pad."""
            nc.vector.tensor_tensor(out=out_t[:, :, 0:W - 1], in0=in_t[:, :, 0:W - 1],
                                    in1=in_t[:, :, 1:W], op=op)
            nc.vector.tensor_copy(out_t[:, :, W - 1:W], in_t[:, :, W - 1:W])
            nc.vector.tensor_tensor(out=out_t[:, :, 1:W], in0=out_t[:, :, 1:W],
                                    in1=in_t[:, :, 0:W - 1], op=op)

        # ---- software-pipelined emission: pools(b+1) are emitted before
        # vmax(b) so the DVE never idles at its FIFO head waiting on shift DMAs
        def stage1(b):
            pl0 = BP * b
            xs = {}
            for h, r0 in ((0, 0), (1, 128)):
                X = ld_pool.tile([128, BP, W], BF16, tag=f"x{h}", name=f"x_{b}_{h}")
                nc.gpsimd.dma_start(
                    out=X[:], in_=x_ap[pl0:pl0 + BP, r0:r0 + 128, :].transpose([1, 0, 2]))
                xs[h] = X
            hms, hss = {}, {}
            for h in (0, 1):
                hm = hm_pool.tile([128, BP, W], BF16, tag=f"hm{h}", name=f"hm_{b}_{h}")
                hpool(hm, xs[h], MAXOP)
                hs = hm_pool.tile([128, BP, W], BF16, tag=f"hs{h}", name=f"hs_{b}_{h}")
                hpool(hs, xs[h], ADDOP)
                hms[h], hss[h] = hm, hs
            # shift copies can be emitted now (they only need hm)
            uds = {}
            for h in (0, 1):
                U = ud_pool.tile([128, BP, W], BF16, tag=f"u{h}", name=f"u_{b}_{h}")
                nc.gpsimd.dma_start(out=U[0:127], in_=hms[h][1:128])
                if h == 0:
                    nc.sync.dma_start(out=U[127:128], in_=hms[1][0:1])
                else:
                    nc.sync.dma_start(out=U[127:128], in_=hms[1][127:128])
                D = ud_pool.tile([128, BP, W], BF16, tag=f"d{h}", name=f"d_{b}_{h}")
                nc.gpsimd.dma_start(out=D[1:128], in_=hms[h][0:127])
                if h == 0:
                    nc.sync.dma_start(out=D[0:1], in_=hms[0][0:1])
                else:
                    nc.sync.dma_start(out=D[0:1], in_=hms[0][127:128])
                uds[h] = (U, D)
            return xs, hms, hss, uds

        def stage2(b, st):
            xs, hms, hss, uds = st
            # vmax in place: hm becomes the pooled max tile
            mxs = hms
            for h in (0, 1):
                U, D = uds[h]
                nc.vector.tensor_tensor(out=hms[h][:], in0=hms[h][:], in1=U[:], op=MAXOP)
                nc.vector.tensor_tensor(out=hms[h][:], in0=hms[h][:], in1=D[:], op=MAXOP)

            # B' frames (rows 124..251)
            mxp = frm_pool.tile([128, BP, W], BF16, tag="mxp", name=f"mxp_{b}")
            nc.sync.dma_start(out=mxp[0:4], in_=mxs[0][124:128])
            nc.sync.dma_start(out=mxp[4:128], in_=mxs[1][0:124])
            hsp = frm_pool.tile([128, BP, W], BF16, tag="hsp", name=f"hsp_{b}")
            nc.sync.dma_start(out=hsp[0:4], in_=hss[0][124:128])
            nc.sync.dma_start(out=hsp[4:128], in_=hss[1][0:124])

            # C-chunk gathers: pooled rows 248..255 -> DRAM -> packed crhs
            nc.sync.dma_start(out=cscr[b, 0], in_=mxs[1][120:128, :, :])
            nc.sync.dma_start(out=cscr[b, 1], in_=hss[1][120:128, :, :])
            for p_loc in range(2):
                p = 2 * b + p_loc
                for pa in range(2):
                    nc.sync.dma_start(
                        out=crhs[48 * pa:48 * pa + 48, p, :],
                        in_=cscr[b, pa, :, 6 * p_loc:6 * p_loc + 6, :].transpose([1, 0, 2]))
            return mxs, hss, mxp, hsp

        def stage3(b, st2):
            mxs, hss, mxp, hsp = st2
            pl0 = BP * b
            # ---- conv: chunks A and B', 2 timestep-pairs per block
            epi = []
            for chunk in range(2):
                M = MA if chunk == 0 else MB
                matbase = 18 * chunk
                pss = [psum_pool.tile([128, 2, W], F32, tag="ps", name=f"ps_{b}_{chunk}_{pp}")
                       for pp in range(2)]
                srcs = (mxs[0], hss[0]) if chunk == 0 else (mxp, hsp)
                n = len(MATS)
                for i, (path, c, kw) in enumerate(MATS):
                    mat = matbase + _mat_index(path, c, kw)
                    s = kw - 1
                    lo, hi = max(0, -s), W - max(0, s)
                    lhsT = cst[0:128, mat, 0:M]
                    # rhs planes (3*tt+c) for tt in {2p, 2p+1}: view (tt, c)
                    rsrc = srcs[path][:].rearrange("k (tt c) w -> k tt c w", tt=4, c=3)
                    for p_loc in range(2):
                        rhs = rsrc[0:128, 2 * p_loc:2 * p_loc + 2, c, lo + s:hi + s]
                        nc.tensor.matmul(pss[p_loc][0:M, :, lo:hi], lhsT, rhs,
                                         start=(i == 0), stop=(i == n - 1))
                sg = sg_pool.tile([126, 4, W], F32, tag=f"sg{chunk}")
                epi.append((chunk, M, pss, sg))
            # epilogue: all Lrelus, then all Sigmoids (avoid ACT table thrash)
            vs = {}
            for chunk, M, pss, sg in epi:
                for p_loc in range(2):
                    v = v_pool.tile([126, 2, W], BF16, tag=f"v{chunk}{p_loc}",
                                    name=f"v_{b}_{chunk}_{p_loc}")
                    nc.scalar.activation(v[0:M], pss[p_loc][0:M],
                                         LRELU, bias=bias[0:M], scale=1.0, alpha=0.01)
                    vs[(chunk, p_loc)] = v
            for chunk, M, pss, sg in epi:
                for p_loc in range(2):
                    nc.scalar.activation(sg[0:M, 2 * p_loc:2 * p_loc + 2, :],
                                         vs[(chunk, p_loc)][0:M], SIGMOID)
            for chunk, M, pss, sg in epi:
                r0 = 0 if chunk == 0 else MA
                dst = out_ap[pl0:pl0 + BP, r0:r0 + M, :].rearrange(
                    "(t c) m w -> m t c w", t=4, c=3)
                for ch in range(3):
                    nc.scalar.dma_start(out=dst[:, :, ch, :], in_=sg[0:M])

            # C matmuls for this block's two pairs
            for p_loc in range(2):
                p = 2 * b + p_loc
                for i, kw in enumerate((1, 0, 2)):
                    s = kw - 1
                    lo, hi = max(0, -s), W - max(0, s)
                    nc.tensor.matmul(psc_slice(p, lo, hi),
                                     cst[0:96, 36 + kw, 0:12],
                                     crhs[0:96, p, lo + s:hi + s],
                                     start=(i == 0), stop=(i == 2))

        st1 = {0: stage1(0)}
        st1[1] = stage1(1)
        st2s = {}
        for b in range(NBLK):
            if b + 2 < NBLK:
                st1[b + 2] = stage1(b + 2)
            st2s[b] = stage2(b, st1.pop(b))
            stage3(b, st2s.pop(b))

        # C epilogue + output (rows 250..255, all t).  ACT needs 32-aligned
        # partition bases -> process the (base, slot) psc layout wholesale,
        # then write out directly with per-(pair, channel) DMAs.
        outv = out_ap.rearrange("(pl three) h w -> pl three h w", pl=16, three=3)
        for ti in range(2):
            vca = const_pool.tile([96, 2, W], BF16, tag=f"vca{ti}", name=f"vca{ti}")
            nc.scalar.activation(vca[:], ps_cs[ti][0:96], LRELU,
                                 bias=bias[0:96], scale=1.0, alpha=0.01)
            sga = const_pool.tile([96, 2, W], F32, tag=f"sga{ti}", name=f"sga{ti}")
            nc.scalar.activation(sga[:], vca[:], SIGMOID)
            for u in range(3 if ti == 0 else 1):
                for slot in range(2):
                    p = 6 * ti + 2 * u + slot
                    for ch in range(3):
                        nc.sync.dma_start(
                            out=outv[2 * p:2 * p + 2, ch, 250:256, :],
                            in_=sga[32 * u:32 * u + 12, slot, :])

    nc.compile()
    return nc


def kernel(input_tensor, conv_w, conv_b):
    input_tensor = np.ascontiguousarray(np.asarray(input_tensor, dtype=np.float32))
    conv_w = np.asarray(conv_w, dtype=np.float32)
    conv_b = np.asarray(conv_b, dtype=np.float32)

    if "nc" not in _cache:
        _cache["nc"] = _build_program()
    nc = _cache["nc"]

    stack = _build_stack(conv_w)
    bias_vec = np.full((128, 1), conv_b[0], dtype=np.float32)
    in_maps = [
        {"x": input_tensor[i], "cst": stack, "bias": bias_vec}
        for i in range(N_CORES)
    ]
    res = run_bass_kernel_spmd(nc, in_maps, list(range(N_CORES)))
    out = np.stack([res.results[i]["out"] for i in range(N_CORES)], axis=0)
    return out.astype(np.float32)


if __name__ == "__main__":
    rng = np.random.default_rng(0)
    x = rng.standard_normal((B, CTOT, H, W), dtype=np.float32)
    cw = rng.uniform(-0.1, 0.1, (1, 6, 3, 3)).astype(np.float32)
    cb = np.array([0.01], dtype=np.float32)
    o = kernel(x, cw, cb)
    print(o.shape, o.dtype)
